# revision 50
# baseline (speedup 1.0000x reference)
"""Trainium2 Bass kernel for nn_Attention_Embedding (spatial NxN attention +
channel CxC attention + conv3d(1,1,4) embedding head).

Sharding: 8 cores = 4 samples x 2 halves (split on H). Each core holds its
sample's full q (softmax rows are complete) and produces its own slice of the
final output; no cross-core communication.

Per core (sample b, local columns i in [0, 2048), all j in [0, 4096)):
  S_T[j,i] = sum_c q[j,c] q[i,c]: bf16 matmuls trio-packed over 3 j-chunks
    (K=32 row bands 0/32/64) in [128,512] psum tiles (6-slot ring).
  P_T = exp(S_T) split across engines by j-chunk parity:
    ACT native exp (psum f32 -> sbuf bf16) for even jc + {13,27};
    DVE Schraudolph for the rest: i16 = trunc(S*184.665 + 16250.5); the
    int16 bit pattern IS bf16(exp(S)) to ~3% (bitcast as the AV rhs).
    Softmax normalization + the small beta scale bury the approximation
    (6.6e-5 end-to-end, validated in numpy and on HW).
  AV: lhsT = [q_aug | (1/beta)-ones] bf16; even jc accumulate into av rows
    0:64, odd into 64:128 (col-pair packing; PE output partitions must equal
    the tile_position column). The ones block yields 32 pre-broadcast
    denominator rows per parity, pre-divided by beta; the Gram picks up the
    same 1/beta factor on its augmented row/col, compensated exactly on the
    host by scaling the wq/wk bias rows.
  Columns run in two 1024-wide pairs; pair-0's fold/normalize chain (ACT
    partition-moving copies + DVE adds/recip) is deferred into pair-1's
    groups, AV emission lags exps by one group, and the channel branch
    (packed bf16 Gram, energy softmax, pa) borrows the psAV slot / "s"
    slots inside pair 0.
  Head: conv3d(1,1,4) over xb (rows caF|paF|caF'|paF', one 516-wide padded
    window per 512-col chunk): pair-0 as one 4-band quad-packed pass that
    runs while pair-1's fold drains; pair-1 as per-chunk 2-band passes
    pipelined behind their own normalize chunks; ACT relu + DVE band-sum,
    DVE 32x32 transpose, strided DMA out. Host drops d' >= 13.
"""

import os
import sys

for _p in ("/opt/trn_rl_repo", "/root/.axon_site/_ro/trn_rl_repo"):
    if os.path.isdir(_p) and _p not in sys.path:
        sys.path.insert(0, _p)
        break

import ml_dtypes
import numpy as np

import concourse.bacc as bacc
import concourse.bass as bass
import concourse.mybir as mybir
import concourse.tile as tile
from concourse import bass_utils

B, H, W, D, C = 4, 16, 16, 16, 32
N = H * W * D            # 4096
NL = N // 2              # 2048 columns per core
DO = D - 3               # 13 conv output positions
NCORES = 8
NJC = N // 128           # 32 j-chunks
NP = 2                   # column pairs (1024 each)
PW = NL // NP            # 1024

f32 = mybir.dt.float32
f32r = mybir.dt.float32r
bf16 = mybir.dt.bfloat16
i16 = mybir.dt.int16
FT = mybir.ActivationFunctionType
ALU = mybir.AluOpType
PSUM = bass.MemorySpace.PSUM

# Schraudolph constants for bf16-pattern exp in int16:
#   i16 = trunc(S * 128/ln2 + (127*128 - 6 + 0.5)), bitcast -> bf16
EXP_A = 184.665024
EXP_C = 16250.5
ACT_ODD = {13, 27}      # odd j-chunks offloaded back to ACT (engine balance)


def _emit(tc, nc, t, out_d):
    with (
        tc.tile_pool(name="const", bufs=1) as cp,
        tc.tile_pool(name="work", bufs=1) as wp,
    ):
        qTP_r = cp.tile([96, N], bf16)           # q^T replicated on 3 row bands
        qTloc_r = cp.tile([C + 1, NL], f32r)
        qTloc_f = cp.tile([C, NL], f32)
        qc2_b = cp.tile([128, NJC, 128], bf16)   # AV lhsT: [data|ones] x2
        wq_f = cp.tile([C + 1, C], f32)
        wk_f = cp.tile([C + 1, C], f32)
        wvT_r = cp.tile([C, C + 1], f32r)
        wcb4 = cp.tile([128, 4 * C], f32r)       # conv weights, 4 bands
        bcb4_v = cp.tile([C, 4], f32)            # conv bias, 1 col per band
        pa_sb = cp.tile([C, NL], f32)            # paF staging (rows 0:32)
        gamma_v = cp.tile([C, 1], f32)
        id32_r = cp.tile([C, C], f32r)
        # conv input rows: caF|paF|caF'|paF'; each 512-col chunk owns a
        # 516-wide window (512 data + 4 pad) so no conv window AP ever
        # crosses chunks
        CW = 516
        xb = cp.tile([128, 4 * CW], f32r)

        # loads: every qT band replica is an independent DRAM read (no
        # sbuf-sbuf chaining -> no sem round-trips on the critical path);
        # qc2 (host-prepped) leads the gpsimd queue for the Gram warmup
        nc.sync.dma_start(qTP_r[0:C, 0:NL], t["qT"][:, 0:NL])
        nc.sync.dma_start(qTP_r[C:2 * C, 0:NL], t["qT"][:, 0:NL])
        nc.sync.dma_start(qTP_r[2 * C:3 * C, 0:NL], t["qT"][:, 0:NL])
        nc.gpsimd.dma_start(qc2_b[:, 0:NJC // 2, :], t["qc2full"][:, 0:NJC // 2, :])
        nc.gpsimd.dma_start(qc2_b[:, NJC // 2:, :], t["qc2full"][:, NJC // 2:, :])
        nc.gpsimd.dma_start(qTP_r[0:C, NL:N], t["qT"][:, NL:N])
        nc.gpsimd.dma_start(qTP_r[C:2 * C, NL:N], t["qT"][:, NL:N])
        nc.gpsimd.dma_start(qTP_r[2 * C:3 * C, NL:N], t["qT"][:, NL:N])
        for name, tl in [
            ("wq", wq_f), ("wk", wk_f), ("wvT", wvT_r), ("id32r", id32_r),
            ("qTloc", qTloc_r), ("qTlocf", qTloc_f),
            ("wcb4", wcb4), ("bcb4", bcb4_v),
            ("gamma", gamma_v),
        ]:
            nc.sync.dma_start(tl[:], t[name])
        # ACT exp table load off the critical path
        warm = wp.tile([1, 1], f32)
        nc.vector.memset(warm[:], 0.0)
        nc.scalar.activation(warm[:], warm[:], FT.Exp)
        nc.vector.memset(
            xb[:].bitcast(f32).rearrange("p (c w) -> p c w", w=CW)[:, :, 512:516],
            0.0,
        )

        with (
            tc.tile_pool(name="psS", bufs=7, space=PSUM) as psS,
            tc.tile_pool(name="psAV", bufs=1, space=PSUM) as psAV,
            tc.tile_pool(name="ptp", bufs=9) as ptp,
            tc.tile_pool(name="pmp", bufs=9) as pmp,
            tc.tile_pool(name="nrm", bufs=2) as nrm,
        ):
            # ---- channel branch part 1 (emitted after S-group-0 so the
            # steady phase starts first; borrows the psAV slot, which the
            # av accumulator claims afterwards) ----
            chan = {}

            def emit_channel_part1():
                # bf16 Gram packed 4-wide: per jc, data rows (M=32) and ones
                # rows (M=2); even jc -> partitions 0:34 (PE cols 0/32), odd
                # -> 64:98 (cols 64/96); PE output partitions must equal the
                # tile_position column. bf16 matmul operands need even free
                # sizes: rhs is 34 wide, ones lhsT 2 cols (duplicates unused).
                g_ps = psAV.tile([98, C + 2], f32, tag="av")
                for jc in range(NJC):
                    po = (jc % 2) * 64
                    nc.tensor.matmul(
                        g_ps[po:po + C, :], qc2_b[:, jc, 0:C], qc2_b[:, jc, 0:C + 2],
                        start=(jc < 2), stop=(jc >= NJC - 2),
                        tile_position=(0, po), skip_group_check=True,
                    )
                    nc.tensor.matmul(
                        g_ps[po + C:po + C + 2, :], qc2_b[:, jc, C:C + 2],
                        qc2_b[:, jc, 0:C + 2],
                        start=(jc < 2), stop=(jc >= NJC - 2),
                        tile_position=(0, po + 32), skip_group_check=True,
                    )
                godd = wp.tile([C + 2, C + 2], f32)
                nc.scalar.copy(godd[:], g_ps[64:98, :])
                g_sb = wp.tile([C + 2, C + 2], f32)
                nc.vector.tensor_tensor(
                    g_sb[:], g_ps[0:34, :], godd[:], op=ALU.add
                )
                t1_ps = psAV.tile([C + 1, C], f32, tag="av")
                nc.tensor.matmul(t1_ps[:], g_sb[0:C + 1, 0:C + 1], wk_f[:], start=True, stop=True)
                t1_sb = wp.tile([C + 1, C], f32)
                nc.vector.tensor_copy(t1_sb[:], t1_ps[:])
                e2_ps = psAV.tile([C, C], f32, tag="av")
                nc.tensor.matmul(e2_ps[:], wq_f[:], t1_sb[:], start=True, stop=True)
                # attn2 = softmax over free; energy2 spans ~[-290, 290]
                mx = wp.tile([C, 1], f32)
                nc.vector.reduce_max(mx[:], e2_ps[:], axis=mybir.AxisListType.X)
                nmx = wp.tile([C, 1], f32)
                nc.vector.tensor_scalar_mul(nmx[:], mx[:], -1.0)
                a_sb = wp.tile([C, C], f32)
                nc.scalar.activation(a_sb[:], e2_ps[:], FT.Exp, bias=nmx[:])
                sm = wp.tile([C, 1], f32)
                nc.vector.reduce_sum(sm[:], a_sb[:], axis=mybir.AxisListType.X)
                rc = wp.tile([C, 1], f32)
                nc.vector.reciprocal(rc[:], sm[:])
                a_n = wp.tile([C, C], f32r)
                nc.vector.tensor_scalar_mul(a_n[:], a_sb[:], rc[:])
                chan["a_n"] = a_n

            # ---- steady phase: S -> exp -> AV over column pairs ----
            def emit_channel_part2():
                # attn2^T, wpa = wv_aug @ attn2^T, pa_T = wpa @ qloc_aug^T;
                # borrows "s" slots mid-pair-0 (DVE has slack there)
                at_ps = psS.tile([C, C], f32, tag="s")
                nc.tensor.matmul(at_ps[:], chan["a_n"][:], id32_r[:], start=True, stop=True)
                at_r = wp.tile([C, C], f32r)
                nc.vector.tensor_copy(at_r[:], at_ps[:])
                wpa_ps = psS.tile([C + 1, C], f32, tag="s")
                nc.tensor.matmul(wpa_ps[:], wvT_r[:], at_r[:], start=True, stop=True)
                wpa_r = wp.tile([C + 1, C], f32r)
                nc.vector.tensor_copy(wpa_r[:], wpa_ps[:])
                for gg in range(NP):
                    for h in range(2):
                        c0 = gg * PW + h * 512
                        pa_ps = psS.tile([C, 512], f32, tag="s", name=f"pa{gg}{h}")
                        nc.tensor.matmul(
                            pa_ps[:], wpa_r[:], qTloc_r[:, c0:c0 + 512],
                            start=True, stop=True,
                        )
                        nc.vector.scalar_tensor_tensor(
                            pa_sb[:, c0:c0 + 512], pa_ps[:],
                            gamma_v[:], qTloc_f[:, c0:c0 + 512],
                            op0=ALU.mult, op1=ALU.add,
                        )
                # paF into xb rows 32:64 (strided DMA does the partition
                # move and the per-chunk window scatter)
                nc.sync.dma_start(
                    xb[C:2 * C, :].rearrange("p (c w) -> p c w", w=CW)[:, :, 0:512],
                    pa_sb[:].bitcast(f32r).rearrange("p (c w) -> p c w", w=512),
                )

            deferred = []
            for p in range(NP):
                cols = slice(p * PW, (p + 1) * PW)
                av_cell = [None]
                if p > 0:
                    av_cell[0] = psAV.tile([128, 512], f32, tag="av", name=f"av{p}")
                pt_of = {}
                next_pair = [0]

                def emit_av_ready(limit, av_cell=av_cell, pt_of=pt_of,
                                  next_pair=next_pair):
                    # per jc: the two 512-col half-chunks pack at PE columns
                    # 0/64 -> av rows 0:64 / 64:128, each [data|denominator];
                    # no parity split, so no cross-partition fold is needed
                    while next_pair[0] < limit:
                        jc = next_pair[0]
                        av = av_cell[0]
                        for h in range(2):
                            pt = pt_of.pop((jc, h))
                            rhs = pt[:]
                            if rhs.dtype == i16:
                                rhs = rhs.bitcast(bf16)
                            nc.tensor.matmul(
                                av[64 * h:64 * h + 64, :],
                                qc2_b[:, jc, 64 * h:64 * h + 64], rhs,
                                start=(jc == 0), stop=(jc == NJC - 1),
                                tile_position=(0, 64 * h), skip_group_check=True,
                            )
                        next_pair[0] += 1

                for gi, g0 in enumerate(range(0, NJC, 3)):
                    trio = range(g0, min(g0 + 3, NJC))
                    for h in range(2):
                        s_tiles = []
                        c0 = p * PW + h * 512
                        for r, jc in enumerate(trio):
                            s_ps = psS.tile([128, 512], f32, tag="s", name=f"s_{p}_{g0}_{r}_{h}")
                            nc.tensor.matmul(
                                s_ps[:],
                                qTP_r[32 * r:32 * r + C, jc * 128:(jc + 1) * 128],
                                qTP_r[32 * r:32 * r + C, c0:c0 + 512],
                                start=True, stop=True,
                                tile_position=(32 * r, 0), skip_group_check=True,
                            )
                            s_tiles.append(s_ps)
                        for r, jc in enumerate(trio):
                            if jc % 2 == 0 or jc in ACT_ODD:
                                pt = ptp.tile([128, 512], bf16, tag="pt")
                                nc.scalar.activation(pt[:], s_tiles[r][:], FT.Exp)
                            else:
                                pt = pmp.tile([128, 512], i16, tag="ptm")
                                nc.vector.tensor_scalar(
                                    pt[:], s_tiles[r][:], EXP_A, EXP_C,
                                    op0=ALU.mult, op1=ALU.add,
                                )
                            pt_of[(jc, h)] = pt
                    if p == 0 and gi == 0:
                        emit_channel_part1()
                    if av_cell[0] is None and g0 >= 2:
                        av_cell[0] = psAV.tile([128, 512], f32, tag="av", name=f"av{p}")
                    # AV lags the exps by one group: both parities' exp tiles
                    # are done, so the packed pair issues back-to-back
                    emit_av_ready(g0)
                    for _ in range(3):
                        if deferred:
                            deferred.pop(0)()
                    if p == 0 and gi == 4:
                        emit_channel_part2()
                emit_av_ready(NJC)
                av = av_cell[0]

                # pair tail: fold even/odd AV halves (ACT copies move the
                # odd/denominator rows to partition base 0; the copies +
                # DVE adds also free the av slot for the next pair), then
                # beta/denominator scale + caF into xb rows 0:32; for pair 0
                # the scale chain is deferred into pair 1's groups so it
                # doesn't delay pair-1 exps.
                avn_e = nrm.tile([C, PW], f32, tag="avn_e")
                recB = nrm.tile([C, PW], f32, tag="recB")

                def chunk_steps(h, av=av, avn_e=avn_e, recB=recB, p=p):
                    c = 2 * p + h
                    xw = slice(c * CW, c * CW + 512)
                    ql = slice(c * 512, (c + 1) * 512)
                    hs = slice(h * 512, (h + 1) * 512)
                    return [
                        lambda: nc.scalar.copy(
                            avn_e[:, hs], av[64 * h + C:64 * h + 2 * C, :]),
                        lambda: nc.vector.reciprocal_approx_fast(
                            recB[:, hs], avn_e[:, hs]),
                        lambda: nc.vector.tensor_tensor(
                            xb[0:C, xw], av[64 * h:64 * h + C, :], recB[:, hs],
                            op=ALU.mult),
                        lambda: nc.vector.tensor_tensor(
                            xb[0:C, xw], xb[0:C, xw], qTloc_f[:, ql],
                            op=ALU.add),
                    ]

                def copy_bands(p=p):
                    nc.sync.dma_start(
                        xb[64:128, 2 * p * CW:2 * (p + 1) * CW],
                        xb[0:64, 2 * p * CW:2 * (p + 1) * CW],
                    )

                if p < NP - 1:
                    deferred += chunk_steps(0) + chunk_steps(1) + [copy_bands]
                else:
                    last_steps = chunk_steps(0) + chunk_steps(1)
            for fn in deferred:
                fn()

            # ---- conv head: pair-0 columns as one 4-band packed pass
            # (runs while pair-1's fold chain drains); pair-1 as per-chunk
            # 2-band passes pipelined behind their own normalize chunks ----
            so = wp.tile([C, NL], f32)
            out_v = out_d.rearrange("(g kk r) f -> g r kk f", kk=16, r=C)
            with tc.tile_pool(name="obp", bufs=4) as obp:

                def finish_chunk(g, cv_ch, cv_po):
                    r0 = wp.tile([C, 512], f32, tag="rt0", name=f"rc{g}")
                    nc.scalar.activation(r0[:], cv_ch, FT.Relu, bias=bcb4_v[:, 0:1])
                    r1 = wp.tile([C, 512], f32, tag="rt1", name=f"rp{g}")
                    nc.scalar.activation(r1[:], cv_po, FT.Relu, bias=bcb4_v[:, 1:2])
                    nc.vector.tensor_tensor(
                        so[:, g * 512:(g + 1) * 512], r0[:], r1[:], op=ALU.add
                    )
                    tb = obp.tile([C, 512], f32, tag="ob", name=f"tb{g}")
                    nc.vector.transpose(tb[:], so[:, g * 512:(g + 1) * 512])
                    eng2 = nc.sync if g % 2 == 0 else nc.gpsimd
                    eng2.dma_start(
                        out_v[g],
                        tb[:].rearrange("r (kk f) -> r kk f", kk=16),
                    )

                # pair-0: 4-band quad-packed (chunks 0,1 x ch/pos)
                cvs = []
                for r in range(4):
                    cv = psS.tile([C, 512], f32, tag="s", name=f"cvA{r}")
                    cvs.append(cv)
                for k in range(4):
                    for r in range(4):
                        ch = r // 2
                        nc.tensor.matmul(
                            cvs[r][:],
                            wcb4[32 * r:32 * r + C, k * C:(k + 1) * C],
                            xb[32 * r:32 * r + C, ch * CW + k:ch * CW + k + 512],
                            start=(k == 0), stop=(k == 3),
                            tile_position=(32 * r, 0), skip_group_check=True,
                        )
                # pair-1 fold/normalize chunks drain onto ACT/DVE here
                for fn in last_steps:
                    fn()
                finish_chunk(0, cvs[0][:], cvs[1][:])
                finish_chunk(1, cvs[2][:], cvs[3][:])
                # pair-1: per-chunk 2-band passes (each waits only its own
                # normalize chunk)
                for c in (2, 3):
                    cvc = psS.tile([C, 512], f32, tag="s", name=f"cvc{c}")
                    cvp = psS.tile([C, 512], f32, tag="s", name=f"cvp{c}")
                    for k in range(4):
                        for r, cv in ((0, cvc), (1, cvp)):
                            nc.tensor.matmul(
                                cv[:],
                                wcb4[32 * r:32 * r + C, k * C:(k + 1) * C],
                                xb[32 * r:32 * r + C, c * CW + k:c * CW + k + 512],
                                start=(k == 0), stop=(k == 3),
                                tile_position=(32 * r, 0), skip_group_check=True,
                            )
                    finish_chunk(c, cvc[:], cvp[:])


def _build():
    nc = bacc.Bacc("TRN2", target_bir_lowering=False, debug=False)
    t = {}

    def din(name, shape, dt):
        t[name] = nc.dram_tensor(name, shape, dt, kind="ExternalInput").ap()

    din("qT", [C, N], bf16)
    din("qTloc", [C + 1, NL], f32r)
    din("qTlocf", [C, NL], f32)
    din("qc2full", [128, NJC, 128], bf16)
    din("wq", [C + 1, C], f32)
    din("wk", [C + 1, C], f32)
    din("wvT", [C, C + 1], f32r)
    din("wcb4", [128, 4 * C], f32r)
    din("bcb4", [C, 4], f32)
    din("gamma", [C, 1], f32)
    din("id32r", [C, C], f32r)
    out_d = nc.dram_tensor("out", [NL, C], f32, kind="ExternalOutput").ap()

    with tile.TileContext(nc) as tc:
        _emit(tc, nc, t, out_d)
    nc.compile()
    return nc


_NC = None


def _get_nc():
    global _NC
    if _NC is None:
        _NC = _build()
    return _NC


def _prepare_in_maps(inputs):
    x = np.asarray(inputs["inputs"], np.float32)
    beta = np.asarray(inputs["beta"], np.float32)
    gamma = np.asarray(inputs["gamma"], np.float32)
    # qc2's ones block holds 1/beta (bf16) so AV denominators absorb the
    # beta scale; the Gram's augmented row/col pick up the same factor,
    # compensated exactly by scaling the wq/wk bias rows by its inverse
    inv_b = np.float32(ml_dtypes.bfloat16(1.0 / beta[0]))
    bscale = np.float32(1.0) / inv_b
    wq_aug = np.concatenate(
        [np.asarray(inputs["wq"], np.float32), np.asarray(inputs["bq"], np.float32)[None, :] * bscale], 0
    )
    wk_aug = np.concatenate(
        [np.asarray(inputs["wk"], np.float32), np.asarray(inputs["bk"], np.float32)[None, :] * bscale], 0
    )
    wv_aug = np.concatenate(
        [np.asarray(inputs["wv"], np.float32), np.asarray(inputs["bv"], np.float32)[None, :]], 0
    )
    wch1 = np.asarray(inputs["w_ch"], np.float32).reshape(4, C, C).transpose(1, 0, 2).reshape(C, 4 * C)
    wpos1 = np.asarray(inputs["w_pos"], np.float32).reshape(4, C, C).transpose(1, 0, 2).reshape(C, 4 * C)
    bch = np.asarray(inputs["b_ch"], np.float32)
    bpos = np.asarray(inputs["b_pos"], np.float32)
    shared = {
        "wq": wq_aug, "wk": wk_aug, "wvT": np.ascontiguousarray(wv_aug.T),
        "wcb4": np.ascontiguousarray(np.concatenate([wch1, wpos1, wch1, wpos1], 0)),
        "bcb4": np.ascontiguousarray(
            np.stack([bch, bpos, bch, bpos], 1)
        ),
        "gamma": np.full((C, 1), gamma[0], np.float32),
        "id32r": np.eye(C, dtype=np.float32),
    }
    in_maps = []
    for core in range(NCORES):
        b, s = core // 2, core % 2
        qs = x[b].reshape(N, C)
        # local-half-first column permutation: S_T rhs slices [0, NL) are the
        # core's own rows; softmax sums over all j are order-invariant.
        q = np.concatenate([qs[s * NL:(s + 1) * NL], qs[(1 - s) * NL:(2 - s) * NL]])
        q_aug = np.concatenate([q, np.ones((N, 1), np.float32)], 1)
        qloc_aug = q_aug[:NL]
        qc = np.ascontiguousarray(q_aug.reshape(NJC, 128, C + 1).transpose(1, 0, 2))
        m = dict(shared)
        m["qT"] = np.ascontiguousarray(q.T).astype(ml_dtypes.bfloat16)
        m["qTloc"] = np.ascontiguousarray(qloc_aug.T)
        m["qTlocf"] = np.ascontiguousarray(qloc_aug.T[:C])
        qc2 = np.empty((128, NJC, 128), np.float32)
        qc2[:, :, 0:C] = qc[:, :, :C]
        qc2[:, :, C:2 * C] = inv_b
        qc2[:, :, 2 * C:] = qc2[:, :, 0:2 * C]
        m["qc2full"] = qc2.astype(ml_dtypes.bfloat16)
        in_maps.append(m)
    return in_maps


def _run(inputs, trace=False):
    nc = _get_nc()
    in_maps = _prepare_in_maps(inputs)
    res = bass_utils.run_bass_kernel_spmd(
        nc, in_maps, core_ids=list(range(NCORES)), trace=trace
    )
    out = np.empty((B, H, W, DO, C), np.float32)
    for core in range(NCORES):
        b, s = core // 2, core % 2
        full = res.results[core]["out"].reshape(8, W, D, C)
        out[b, s * 8:(s + 1) * 8] = full[:, :, :DO, :]
    return out, res


def kernel(**inputs):
    out, _ = _run(inputs, trace=False)
    return out


# revision 51
# speedup vs baseline: 1.1943x; 1.1943x over previous
"""Trainium2 Bass kernel for nn_Attention_Embedding (spatial NxN attention +
channel CxC attention + conv3d(1,1,4) embedding head).

Sharding: 8 cores = 4 samples x 2 halves (split on H). Each core holds its
sample's full q (softmax rows are complete) and produces its own slice of the
final output; no cross-core communication.

Per core (sample b, local columns i in [0, 2048), all j in [0, 4096)):
  S_T[j,i] = sum_c q[j,c] q[i,c]: bf16 matmuls trio-packed over 3 j-chunks
    (K=32 row bands 0/32/64) in [128,512] psum tiles (6-slot ring).
  P_T = exp(S_T) split across engines by j-chunk parity:
    ACT native exp (psum f32 -> sbuf bf16) for even jc + {13,27};
    DVE Schraudolph for the rest: i16 = trunc(S*184.665 + 16250.5); the
    int16 bit pattern IS bf16(exp(S)) to ~3% (bitcast as the AV rhs).
    Softmax normalization + the small beta scale bury the approximation
    (6.6e-5 end-to-end, validated in numpy and on HW).
  AV: lhsT = [q_aug | (1/beta)-ones] bf16; per jc the two 512-col half
    chunks pack at PE columns 0/64 -> av rows 0:64 / 64:128 (PE output
    partitions must equal the tile_position column), each half [data(32) |
    denominator(32)] in one PSUM bank, so no cross-partition fold is ever
    needed: one ACT copy moves each denominator block to partition 0, the
    reciprocal IS beta/sum (ones hold 1/beta), and the caF multiply reads
    the av data rows straight from PSUM. The Gram picks up the same 1/beta
    factor on its augmented row/col, compensated exactly on the host by
    scaling the wq/wk bias rows.
  Columns run in two 1024-wide pairs; pair-0's fold/normalize chain (ACT
    partition-moving copies + DVE adds/recip) is deferred into pair-1's
    groups, AV emission lags exps by one group, and the channel branch
    (packed bf16 Gram, energy softmax, pa) borrows the psAV slot / "s"
    slots inside pair 0.
  Head: conv3d(1,1,4) over xb (rows caF|paF|caF'|paF', one 516-wide padded
    window per 512-col chunk): pair-0 as one 4-band quad-packed pass that
    runs while pair-1's fold drains; pair-1 as per-chunk 2-band passes
    pipelined behind their own normalize chunks; ACT relu + DVE band-sum,
    DVE 32x32 transpose, strided DMA out. Host drops d' >= 13.
"""

import os
import sys

for _p in ("/opt/trn_rl_repo", "/root/.axon_site/_ro/trn_rl_repo"):
    if os.path.isdir(_p) and _p not in sys.path:
        sys.path.insert(0, _p)
        break

import ml_dtypes
import numpy as np

import concourse.bacc as bacc
import concourse.bass as bass
import concourse.mybir as mybir
import concourse.tile as tile
from concourse import bass_utils

B, H, W, D, C = 4, 16, 16, 16, 32
N = H * W * D            # 4096
NL = N // 2              # 2048 columns per core
DO = D - 3               # 13 conv output positions
NCORES = 8
NJC = N // 128           # 32 j-chunks
NP = 2                   # column pairs (1024 each)
PW = NL // NP            # 1024

f32 = mybir.dt.float32
f32r = mybir.dt.float32r
bf16 = mybir.dt.bfloat16
i16 = mybir.dt.int16
FT = mybir.ActivationFunctionType
ALU = mybir.AluOpType
PSUM = bass.MemorySpace.PSUM

# Schraudolph constants for bf16-pattern exp in int16:
#   i16 = trunc(S * 128/ln2 + (127*128 - 6 + 0.5)), bitcast -> bf16
EXP_A = 184.665024
EXP_C = 16250.5
ACT_ODD = {13, 27}      # odd j-chunks offloaded back to ACT (engine balance)


def _emit(tc, nc, t, out_d):
    with (
        tc.tile_pool(name="const", bufs=1) as cp,
        tc.tile_pool(name="work", bufs=1) as wp,
    ):
        qTP_r = cp.tile([96, N], bf16)           # q^T replicated on 3 row bands
        qTloc_r = cp.tile([C + 1, NL], f32r)
        qTloc_f = cp.tile([C, NL], f32)
        qc2_b = cp.tile([128, NJC, 128], bf16)   # AV lhsT: [data|ones] x2
        wq_f = cp.tile([C + 1, C], f32)
        wk_f = cp.tile([C + 1, C], f32)
        wvT_r = cp.tile([C, C + 1], f32r)
        wcb4 = cp.tile([128, 4 * C], f32r)       # conv weights, 4 bands
        bcb4_v = cp.tile([C, 4], f32)            # conv bias, 1 col per band
        pa_sb = cp.tile([C, NL], f32)            # paF staging (rows 0:32)
        gamma_v = cp.tile([C, 1], f32)
        id32_r = cp.tile([C, C], f32r)
        # conv input rows: caF|paF|caF'|paF'; each 512-col chunk owns a
        # 516-wide window (512 data + 4 pad) so no conv window AP ever
        # crosses chunks
        CW = 516
        xb = cp.tile([128, 4 * CW], f32r)

        # loads: every qT band replica is an independent DRAM read (no
        # sbuf-sbuf chaining -> no sem round-trips on the critical path);
        # qc2 (host-prepped) leads the gpsimd queue for the Gram warmup
        nc.sync.dma_start(qTP_r[0:C, 0:NL], t["qT"][:, 0:NL])
        nc.sync.dma_start(qTP_r[C:2 * C, 0:NL], t["qT"][:, 0:NL])
        nc.sync.dma_start(qTP_r[2 * C:3 * C, 0:NL], t["qT"][:, 0:NL])
        nc.gpsimd.dma_start(qc2_b[:, 0:NJC // 2, :], t["qc2full"][:, 0:NJC // 2, :])
        nc.gpsimd.dma_start(qc2_b[:, NJC // 2:, :], t["qc2full"][:, NJC // 2:, :])
        nc.gpsimd.dma_start(qTP_r[0:C, NL:N], t["qT"][:, NL:N])
        nc.gpsimd.dma_start(qTP_r[C:2 * C, NL:N], t["qT"][:, NL:N])
        nc.gpsimd.dma_start(qTP_r[2 * C:3 * C, NL:N], t["qT"][:, NL:N])
        for name, tl in [
            ("wq", wq_f), ("wk", wk_f), ("wvT", wvT_r), ("id32r", id32_r),
            ("qTloc", qTloc_r), ("qTlocf", qTloc_f),
            ("wcb4", wcb4), ("bcb4", bcb4_v),
            ("gamma", gamma_v),
        ]:
            nc.sync.dma_start(tl[:], t[name])
        # ACT exp table load off the critical path
        warm = wp.tile([1, 1], f32)
        nc.vector.memset(warm[:], 0.0)
        nc.scalar.activation(warm[:], warm[:], FT.Exp)
        nc.vector.memset(
            xb[:].bitcast(f32).rearrange("p (c w) -> p c w", w=CW)[:, :, 512:516],
            0.0,
        )

        with (
            tc.tile_pool(name="psS", bufs=7, space=PSUM) as psS,
            tc.tile_pool(name="psAV", bufs=1, space=PSUM) as psAV,
            tc.tile_pool(name="ptp", bufs=9) as ptp,
            tc.tile_pool(name="pmp", bufs=9) as pmp,
            tc.tile_pool(name="nrm", bufs=2) as nrm,
        ):
            # ---- channel branch part 1 (emitted after S-group-0 so the
            # steady phase starts first; borrows the psAV slot, which the
            # av accumulator claims afterwards) ----
            chan = {}

            def emit_channel_part1():
                # bf16 Gram packed 4-wide: per jc, data rows (M=32) and ones
                # rows (M=2); even jc -> partitions 0:34 (PE cols 0/32), odd
                # -> 64:98 (cols 64/96); PE output partitions must equal the
                # tile_position column. bf16 matmul operands need even free
                # sizes: rhs is 34 wide, ones lhsT 2 cols (duplicates unused).
                g_ps = psAV.tile([98, C + 2], f32, tag="av")
                for jc in range(NJC):
                    po = (jc % 2) * 64
                    nc.tensor.matmul(
                        g_ps[po:po + C, :], qc2_b[:, jc, 0:C], qc2_b[:, jc, 0:C + 2],
                        start=(jc < 2), stop=(jc >= NJC - 2),
                        tile_position=(0, po), skip_group_check=True,
                    )
                    nc.tensor.matmul(
                        g_ps[po + C:po + C + 2, :], qc2_b[:, jc, C:C + 2],
                        qc2_b[:, jc, 0:C + 2],
                        start=(jc < 2), stop=(jc >= NJC - 2),
                        tile_position=(0, po + 32), skip_group_check=True,
                    )
                godd = wp.tile([C + 2, C + 2], f32)
                nc.scalar.copy(godd[:], g_ps[64:98, :])
                g_sb = wp.tile([C + 2, C + 2], f32)
                nc.vector.tensor_tensor(
                    g_sb[:], g_ps[0:34, :], godd[:], op=ALU.add
                )
                t1_ps = psAV.tile([C + 1, C], f32, tag="av")
                nc.tensor.matmul(t1_ps[:], g_sb[0:C + 1, 0:C + 1], wk_f[:], start=True, stop=True)
                t1_sb = wp.tile([C + 1, C], f32)
                nc.vector.tensor_copy(t1_sb[:], t1_ps[:])
                e2_ps = psAV.tile([C, C], f32, tag="av")
                nc.tensor.matmul(e2_ps[:], wq_f[:], t1_sb[:], start=True, stop=True)
                # attn2 = softmax over free; energy2 spans ~[-290, 290]
                mx = wp.tile([C, 1], f32)
                nc.vector.reduce_max(mx[:], e2_ps[:], axis=mybir.AxisListType.X)
                nmx = wp.tile([C, 1], f32)
                nc.vector.tensor_scalar_mul(nmx[:], mx[:], -1.0)
                a_sb = wp.tile([C, C], f32)
                nc.scalar.activation(a_sb[:], e2_ps[:], FT.Exp, bias=nmx[:])
                sm = wp.tile([C, 1], f32)
                nc.vector.reduce_sum(sm[:], a_sb[:], axis=mybir.AxisListType.X)
                rc = wp.tile([C, 1], f32)
                nc.vector.reciprocal(rc[:], sm[:])
                a_n = wp.tile([C, C], f32r)
                nc.vector.tensor_scalar_mul(a_n[:], a_sb[:], rc[:])
                chan["a_n"] = a_n

            # ---- steady phase: S -> exp -> AV over column pairs ----
            def emit_channel_part2():
                # attn2^T, wpa = wv_aug @ attn2^T, pa_T = wpa @ qloc_aug^T;
                # borrows "s" slots mid-pair-0 (DVE has slack there)
                at_ps = psS.tile([C, C], f32, tag="s")
                nc.tensor.matmul(at_ps[:], chan["a_n"][:], id32_r[:], start=True, stop=True)
                at_r = wp.tile([C, C], f32r)
                nc.vector.tensor_copy(at_r[:], at_ps[:])
                wpa_ps = psS.tile([C + 1, C], f32, tag="s")
                nc.tensor.matmul(wpa_ps[:], wvT_r[:], at_r[:], start=True, stop=True)
                wpa_r = wp.tile([C + 1, C], f32r)
                nc.vector.tensor_copy(wpa_r[:], wpa_ps[:])
                for gg in range(NP):
                    for h in range(2):
                        c0 = gg * PW + h * 512
                        pa_ps = psS.tile([C, 512], f32, tag="s", name=f"pa{gg}{h}")
                        nc.tensor.matmul(
                            pa_ps[:], wpa_r[:], qTloc_r[:, c0:c0 + 512],
                            start=True, stop=True,
                        )
                        nc.vector.scalar_tensor_tensor(
                            pa_sb[:, c0:c0 + 512], pa_ps[:],
                            gamma_v[:], qTloc_f[:, c0:c0 + 512],
                            op0=ALU.mult, op1=ALU.add,
                        )
                # paF into xb rows 32:64 (strided DMA does the partition
                # move and the per-chunk window scatter)
                nc.sync.dma_start(
                    xb[C:2 * C, :].rearrange("p (c w) -> p c w", w=CW)[:, :, 0:512],
                    pa_sb[:].bitcast(f32r).rearrange("p (c w) -> p c w", w=512),
                )

            deferred = []
            for p in range(NP):
                cols = slice(p * PW, (p + 1) * PW)
                av_cell = [None]
                if p > 0:
                    av_cell[0] = psAV.tile([128, 512], f32, tag="av", name=f"av{p}")
                pt_of = {}
                next_pair = [0]

                def emit_av_ready(limit, av_cell=av_cell, pt_of=pt_of,
                                  next_pair=next_pair):
                    # per jc: the two 512-col half-chunks pack at PE columns
                    # 0/64 -> av rows 0:64 / 64:128, each [data|denominator];
                    # no parity split, so no cross-partition fold is needed
                    while next_pair[0] < limit:
                        jc = next_pair[0]
                        av = av_cell[0]
                        for h in range(2):
                            pt = pt_of.pop((jc, h))
                            rhs = pt[:]
                            if rhs.dtype == i16:
                                rhs = rhs.bitcast(bf16)
                            nc.tensor.matmul(
                                av[64 * h:64 * h + 64, :],
                                qc2_b[:, jc, 64 * h:64 * h + 64], rhs,
                                start=(jc == 0), stop=(jc == NJC - 1),
                                tile_position=(0, 64 * h), skip_group_check=True,
                            )
                        next_pair[0] += 1

                for gi, g0 in enumerate(range(0, NJC, 3)):
                    trio = range(g0, min(g0 + 3, NJC))
                    for h in range(2):
                        s_tiles = []
                        c0 = p * PW + h * 512
                        for r, jc in enumerate(trio):
                            s_ps = psS.tile([128, 512], f32, tag="s", name=f"s_{p}_{g0}_{r}_{h}")
                            nc.tensor.matmul(
                                s_ps[:],
                                qTP_r[32 * r:32 * r + C, jc * 128:(jc + 1) * 128],
                                qTP_r[32 * r:32 * r + C, c0:c0 + 512],
                                start=True, stop=True,
                                tile_position=(32 * r, 0), skip_group_check=True,
                            )
                            s_tiles.append(s_ps)
                        for r, jc in enumerate(trio):
                            if jc % 2 == 0 or jc in ACT_ODD:
                                pt = ptp.tile([128, 512], bf16, tag="pt")
                                nc.scalar.activation(pt[:], s_tiles[r][:], FT.Exp)
                            else:
                                pt = pmp.tile([128, 512], i16, tag="ptm")
                                nc.vector.tensor_scalar(
                                    pt[:], s_tiles[r][:], EXP_A, EXP_C,
                                    op0=ALU.mult, op1=ALU.add,
                                )
                            pt_of[(jc, h)] = pt
                    if p == 0 and gi == 0:
                        emit_channel_part1()
                    if av_cell[0] is None and g0 >= 2:
                        av_cell[0] = psAV.tile([128, 512], f32, tag="av", name=f"av{p}")
                    # AV lags the exps by one group: both parities' exp tiles
                    # are done, so the packed pair issues back-to-back
                    emit_av_ready(g0)
                    for _ in range(3):
                        if deferred:
                            deferred.pop(0)()
                    if p == 0 and gi == 4:
                        emit_channel_part2()
                emit_av_ready(NJC)
                av = av_cell[0]

                # pair tail: fold even/odd AV halves (ACT copies move the
                # odd/denominator rows to partition base 0; the copies +
                # DVE adds also free the av slot for the next pair), then
                # beta/denominator scale + caF into xb rows 0:32; for pair 0
                # the scale chain is deferred into pair 1's groups so it
                # doesn't delay pair-1 exps.
                avn_e = nrm.tile([C, PW], f32, tag="avn_e")
                recB = nrm.tile([C, PW], f32, tag="recB")

                def chunk_steps(h, av=av, avn_e=avn_e, recB=recB, p=p):
                    c = 2 * p + h
                    xw = slice(c * CW, c * CW + 512)
                    ql = slice(c * 512, (c + 1) * 512)
                    hs = slice(h * 512, (h + 1) * 512)
                    return [
                        lambda: nc.scalar.copy(
                            avn_e[:, hs], av[64 * h + C:64 * h + 2 * C, :]),
                        lambda: nc.vector.reciprocal_approx_fast(
                            recB[:, hs], avn_e[:, hs]),
                        lambda: nc.vector.tensor_tensor(
                            xb[0:C, xw], av[64 * h:64 * h + C, :], recB[:, hs],
                            op=ALU.mult),
                        lambda: nc.vector.tensor_tensor(
                            xb[0:C, xw], xb[0:C, xw], qTloc_f[:, ql],
                            op=ALU.add),
                    ]

                def copy_bands(p=p):
                    nc.sync.dma_start(
                        xb[64:128, 2 * p * CW:2 * (p + 1) * CW],
                        xb[0:64, 2 * p * CW:2 * (p + 1) * CW],
                    )

                if p < NP - 1:
                    deferred += chunk_steps(0) + chunk_steps(1) + [copy_bands]
                else:
                    last_steps = chunk_steps(0) + chunk_steps(1)
            for fn in deferred:
                fn()

            # ---- conv head: pair-0 columns as one 4-band packed pass
            # (runs while pair-1's fold chain drains); pair-1 as per-chunk
            # 2-band passes pipelined behind their own normalize chunks ----
            so = wp.tile([C, NL], f32)
            out_v = out_d.rearrange("(g kk r) f -> g r kk f", kk=16, r=C)
            with tc.tile_pool(name="obp", bufs=4) as obp:

                def finish_chunk(g, cv_ch, cv_po):
                    r0 = wp.tile([C, 512], f32, tag="rt0", name=f"rc{g}")
                    nc.scalar.activation(r0[:], cv_ch, FT.Relu, bias=bcb4_v[:, 0:1])
                    r1 = wp.tile([C, 512], f32, tag="rt1", name=f"rp{g}")
                    nc.scalar.activation(r1[:], cv_po, FT.Relu, bias=bcb4_v[:, 1:2])
                    nc.vector.tensor_tensor(
                        so[:, g * 512:(g + 1) * 512], r0[:], r1[:], op=ALU.add
                    )
                    tb = obp.tile([C, 512], f32, tag="ob", name=f"tb{g}")
                    nc.vector.transpose(tb[:], so[:, g * 512:(g + 1) * 512])
                    eng2 = nc.sync if g % 2 == 0 else nc.gpsimd
                    eng2.dma_start(
                        out_v[g],
                        tb[:].rearrange("r (kk f) -> r kk f", kk=16),
                    )

                # pair-0: 4-band quad-packed (chunks 0,1 x ch/pos)
                cvs = []
                for r in range(4):
                    cv = psS.tile([C, 512], f32, tag="s", name=f"cvA{r}")
                    cvs.append(cv)
                for k in range(4):
                    for r in range(4):
                        ch = r // 2
                        nc.tensor.matmul(
                            cvs[r][:],
                            wcb4[32 * r:32 * r + C, k * C:(k + 1) * C],
                            xb[32 * r:32 * r + C, ch * CW + k:ch * CW + k + 512],
                            start=(k == 0), stop=(k == 3),
                            tile_position=(32 * r, 0), skip_group_check=True,
                        )
                # pair-1 fold/normalize chunks drain onto ACT/DVE here
                for fn in last_steps:
                    fn()
                finish_chunk(0, cvs[0][:], cvs[1][:])
                finish_chunk(1, cvs[2][:], cvs[3][:])
                # pair-1: per-chunk 2-band passes (each waits only its own
                # normalize chunk)
                for c in (2, 3):
                    cvc = psS.tile([C, 512], f32, tag="s", name=f"cvc{c}")
                    cvp = psS.tile([C, 512], f32, tag="s", name=f"cvp{c}")
                    for k in range(4):
                        for r, cv in ((0, cvc), (1, cvp)):
                            nc.tensor.matmul(
                                cv[:],
                                wcb4[32 * r:32 * r + C, k * C:(k + 1) * C],
                                xb[32 * r:32 * r + C, c * CW + k:c * CW + k + 512],
                                start=(k == 0), stop=(k == 3),
                                tile_position=(32 * r, 0), skip_group_check=True,
                            )
                    finish_chunk(c, cvc[:], cvp[:])


def _build():
    nc = bacc.Bacc("TRN2", target_bir_lowering=False, debug=False)
    t = {}

    def din(name, shape, dt):
        t[name] = nc.dram_tensor(name, shape, dt, kind="ExternalInput").ap()

    din("qT", [C, N], bf16)
    din("qTloc", [C + 1, NL], f32r)
    din("qTlocf", [C, NL], f32)
    din("qc2full", [128, NJC, 128], bf16)
    din("wq", [C + 1, C], f32)
    din("wk", [C + 1, C], f32)
    din("wvT", [C, C + 1], f32r)
    din("wcb4", [128, 4 * C], f32r)
    din("bcb4", [C, 4], f32)
    din("gamma", [C, 1], f32)
    din("id32r", [C, C], f32r)
    out_d = nc.dram_tensor("out", [NL, C], f32, kind="ExternalOutput").ap()

    with tile.TileContext(nc) as tc:
        _emit(tc, nc, t, out_d)
    nc.compile()
    return nc


_NC = None


def _get_nc():
    global _NC
    if _NC is None:
        _NC = _build()
    return _NC


def _prepare_in_maps(inputs):
    x = np.asarray(inputs["inputs"], np.float32)
    beta = np.asarray(inputs["beta"], np.float32)
    gamma = np.asarray(inputs["gamma"], np.float32)
    # qc2's ones block holds 1/beta (bf16) so AV denominators absorb the
    # beta scale; the Gram's augmented row/col pick up the same factor,
    # compensated exactly by scaling the wq/wk bias rows by its inverse
    inv_b = np.float32(ml_dtypes.bfloat16(1.0 / beta[0]))
    bscale = np.float32(1.0) / inv_b
    wq_aug = np.concatenate(
        [np.asarray(inputs["wq"], np.float32), np.asarray(inputs["bq"], np.float32)[None, :] * bscale], 0
    )
    wk_aug = np.concatenate(
        [np.asarray(inputs["wk"], np.float32), np.asarray(inputs["bk"], np.float32)[None, :] * bscale], 0
    )
    wv_aug = np.concatenate(
        [np.asarray(inputs["wv"], np.float32), np.asarray(inputs["bv"], np.float32)[None, :]], 0
    )
    wch1 = np.asarray(inputs["w_ch"], np.float32).reshape(4, C, C).transpose(1, 0, 2).reshape(C, 4 * C)
    wpos1 = np.asarray(inputs["w_pos"], np.float32).reshape(4, C, C).transpose(1, 0, 2).reshape(C, 4 * C)
    bch = np.asarray(inputs["b_ch"], np.float32)
    bpos = np.asarray(inputs["b_pos"], np.float32)
    shared = {
        "wq": wq_aug, "wk": wk_aug, "wvT": np.ascontiguousarray(wv_aug.T),
        "wcb4": np.ascontiguousarray(np.concatenate([wch1, wpos1, wch1, wpos1], 0)),
        "bcb4": np.ascontiguousarray(
            np.stack([bch, bpos, bch, bpos], 1)
        ),
        "gamma": np.full((C, 1), gamma[0], np.float32),
        "id32r": np.eye(C, dtype=np.float32),
    }
    in_maps = []
    for core in range(NCORES):
        b, s = core // 2, core % 2
        qs = x[b].reshape(N, C)
        # local-half-first column permutation: S_T rhs slices [0, NL) are the
        # core's own rows; softmax sums over all j are order-invariant.
        q = np.concatenate([qs[s * NL:(s + 1) * NL], qs[(1 - s) * NL:(2 - s) * NL]])
        q_aug = np.concatenate([q, np.ones((N, 1), np.float32)], 1)
        qloc_aug = q_aug[:NL]
        qc = np.ascontiguousarray(q_aug.reshape(NJC, 128, C + 1).transpose(1, 0, 2))
        m = dict(shared)
        m["qT"] = np.ascontiguousarray(q.T).astype(ml_dtypes.bfloat16)
        m["qTloc"] = np.ascontiguousarray(qloc_aug.T)
        m["qTlocf"] = np.ascontiguousarray(qloc_aug.T[:C])
        qc2 = np.empty((128, NJC, 128), np.float32)
        qc2[:, :, 0:C] = qc[:, :, :C]
        qc2[:, :, C:2 * C] = inv_b
        qc2[:, :, 2 * C:] = qc2[:, :, 0:2 * C]
        m["qc2full"] = qc2.astype(ml_dtypes.bfloat16)
        in_maps.append(m)
    return in_maps


def _run(inputs, trace=False):
    nc = _get_nc()
    in_maps = _prepare_in_maps(inputs)
    res = bass_utils.run_bass_kernel_spmd(
        nc, in_maps, core_ids=list(range(NCORES)), trace=trace
    )
    out = np.empty((B, H, W, DO, C), np.float32)
    for core in range(NCORES):
        b, s = core // 2, core % 2
        full = res.results[core]["out"].reshape(8, W, D, C)
        out[b, s * 8:(s + 1) * 8] = full[:, :, :DO, :]
    return out, res


def kernel(**inputs):
    out, _ = _run(inputs, trace=False)
    return out
